# revision 14
# baseline (speedup 1.0000x reference)
"""DEMA (double exponential moving average) Trainium2 kernel, v5.

Math: the per-lane recurrence
    s_t = a*x_t + (1-a)*(s_{t-1} + b_{t-1})
    b_t = B*(s_t - s_{t-1}) + (1-B)*b_{t-1}
is a stable linear filter: the transition matrix A has |eig| = sqrt(0.7)
~= 0.837, so the impulse response decays geometrically.  Instead of
carrying state between chunks (which serializes the scan), every chunk
of C=104 outputs is computed INDEPENDENTLY from a 128-row input window
that includes H=24 rows of history:
    out[t0 .. t0+C) = M @ x[t0-H .. t0-H+128).
The truncated history contributes ~3.4e-3 relative error (gate is 2e-2).
One constant [128,128] matrix M serves all steady chunks; chunk 0
(outputs 1..127) uses an exact matrix G0 that folds in the s_0 = x_0,
b_0 = x_1 - x_0 initialization.  No cross-chunk dependencies exist: the
kernel is a pure streaming pipeline at the DMA roofline.  (A variant
reading the history in place via matmul operands at base partition 64
crashes TRN2 at scale -- the PE needs base partition 0 -- so the H rows
are simply re-read from HBM: +23% input traffic, all operands base 0.)

Precision: all device-side I/O and matmul operands are float16 (PSUM
accumulates in fp32); fp16 quantization adds ~3e-4 relative error.  The
host casts f32->f16 on entry and back on exit.

Sharding: pure data parallel over batch, 4 batches per core x 8 cores.

Engines and scheduling:
  SP ring      input DMAs; 6 grhs slots per batch give ~5 groups of
               prefetch so the input stream never starves and the tail
               drain overlaps compute.
  PE           one 128-contraction matmul per chunk; both chunks of a
               batch's pair are emitted adjacently so the pair copy can
               fire as soon as that batch's matmuls finish (the 2-bank
               PSUM would otherwise lockstep all batches).
  ACT + DVE    PSUM->SBUF pair copies (f32->f16 cast), alternating per
               (pair, batch) so neither engine is the wall; per-engine
               copy semaphores keep cross-engine completion order sound.
  Pool + ACT   output DMAs, one per pair, alternating rings (SWDGE
               descriptor generation on one ring alone would saturate).
All synchronization is explicit single-wait semaphores (at most one sem
wait per instruction; redundant waits are elided).  PSUM: one [128, 2F]
tensor (2 banks) per batch, chunk parity picks the bank, one copy drains
both banks into adjacent gout columns.
"""

import math
from contextlib import ExitStack

import numpy as np

import concourse.bass as bass
from concourse import mybir
from concourse.ap import AP
from concourse.bass_utils import run_bass_kernel_spmd

ALPHA = 0.3
BETA = 0.1

B, T, F = 32, 4096, 512
NCORES = 8
BLOC = B // NCORES
C = 104  # outputs per steady chunk
H = 24   # history rows per chunk window (window = H + C = 128 rows)
G = 4    # chunks per grouped DMA / buffer slot (even: pairs don't span groups)

F16 = mybir.dt.float16
F32 = mybir.dt.float32


def _build_gmats(dtype=np.float16):
    """Return gw [2,128,128]: lhsT weight matrices (transposed, fp16).

    gw[0] = G0.T (chunk 0, exact): out col i (=output t=i+1) from window
    x_0..x_127 with the (s_0, b_0) init folded in; col 127 unused.
    gw[1] = M.T (steady): out col i (=output t_start+i) from window
    x_{t_start-H} .. x_{t_start-H+127}; M[i,u] = h[H+i-u], cols C..127
    unused.  h is the DEMA impulse response of the s output.
    """
    A = np.array(
        [[1 - ALPHA, 1 - ALPHA], [-ALPHA * BETA, BETA * (1 - ALPHA) + 1 - BETA]],
        dtype=np.float64,
    )
    c = np.array([ALPHA, ALPHA * BETA], dtype=np.float64)
    n = H + C + 130
    h = np.zeros(n)
    h[0] = ALPHA
    Pk = [np.eye(2)]
    for k in range(1, n):
        Pk.append(Pk[-1] @ A)
        h[k] = (Pk[k] @ c)[0]

    M = np.zeros((128, 128))
    for i in range(C):
        for u in range(128):
            k = H + i - u
            if k >= 0:
                M[i, u] = h[k]

    G0 = np.zeros((128, 128))
    for t in range(1, 128):
        G0[t - 1, 0] = Pk[t][0, 0] - Pk[t][0, 1]
        G0[t - 1, 1] = Pk[t][0, 1] + h[t - 1]
        for u in range(2, t + 1):
            G0[t - 1, u] = h[t - u]

    return np.ascontiguousarray(np.stack([G0.T, M.T]), dtype=dtype)


def _chunks(t):
    """Steady chunk descriptors: (t0, w0, nout, nwin)."""
    out = []
    t0 = 128
    while t0 < t:
        w0 = t0 - H
        out.append((t0, w0, min(C, t - t0), min(128, t - w0)))
        t0 += C
    return out


def build_nc(bloc=BLOC, t=T, f=F, grp=G, use_dve=True, n_islots=6, n_oslots=3,
             ring_split=True):
    assert grp % 2 == 0
    nc = bass.Bass()
    st = ExitStack()
    nc._dema_exitstack = st  # keep sbuf/psum allocations alive

    chunks = _chunks(t)
    nst = len(chunks)
    groups = [list(range(i, min(i + grp, nst))) for i in range(0, nst, grp)]
    ngrp = len(groups)

    x = nc.dram_tensor("x", [bloc, t, f], F16, kind="ExternalInput")
    gw = nc.dram_tensor("gw", [2, 128, 128], F16, kind="ExternalInput")
    out = nc.dram_tensor("out", [bloc, t, f], F16, kind="ExternalOutput")

    ent = st.enter_context
    wt = ent(nc.sbuf_tensor("wt", [128, 2, 128], F16))
    rhs0 = [ent(nc.sbuf_tensor(f"rhs0_{b}", [128, f], F16)) for b in range(bloc)]
    ot0 = [ent(nc.sbuf_tensor(f"ot0_{b}", [128, f], F16)) for b in range(bloc)]
    grhs = [
        [
            ent(nc.sbuf_tensor(f"grhs_{b}_{s}", [128, grp, f], F16))
            for s in range(n_islots)
        ]
        for b in range(bloc)
    ]
    gout = [
        [
            ent(nc.sbuf_tensor(f"gout_{b}_{s}", [128, grp, f], F16))
            for s in range(n_oslots)
        ]
        for b in range(bloc)
    ]
    # one 2-bank PSUM tensor per batch; chunk m writes bank (m & 1)
    psp = [ent(nc.psum_tensor(f"ps{b}", [128, 2 * f], F32)) for b in range(bloc)]

    s_w = nc.alloc_semaphore("s_w")
    s_in0 = [nc.alloc_semaphore(f"s_in0_{b}") for b in range(bloc)]
    s_ing = [
        [nc.alloc_semaphore(f"s_ing{b}_{s}") for s in range(n_islots)]
        for b in range(bloc)
    ]
    s_o0 = [nc.alloc_semaphore(f"s_o0_{b}") for b in range(bloc)]
    s_og = [
        [nc.alloc_semaphore(f"s_og{b}_{s}") for s in range(n_oslots)]
        for b in range(bloc)
    ]
    s_mm = [nc.alloc_semaphore(f"s_mm{b}") for b in range(bloc)]
    s_cpa = [nc.alloc_semaphore(f"s_cpa{b}") for b in range(bloc)]
    s_cpd = [nc.alloc_semaphore(f"s_cpd{b}") for b in range(bloc)]

    sp, pe, dve, act, pool = nc.sync, nc.tensor, nc.vector, nc.scalar, nc.gpsimd

    all_sems = (
        [s_w]
        + s_in0
        + [s for row in s_ing for s in row]
        + s_o0
        + [s for row in s_og for s in row]
        + s_mm
        + s_cpa
        + s_cpd
    )
    sem_nums = sorted(s.num for s in all_sems)
    lo, hi = sem_nums[0], sem_nums[-1] + 1
    assert sem_nums == list(range(lo, hi))

    # Semaphores are per-core hardware state and are not cleared by
    # allocation: reset ours before any use, and again on exit.
    pool.dma_reset(range(lo, hi))
    pool.sem_clear(range(lo, hi))
    nc.all_engine_barrier()

    # wait elision: engines execute in program order, so a wait for a value
    # <= an earlier wait on the same (engine, sem) is a no-op
    last_wait = {}

    def wait(eng, ename, sem, val):
        key = (ename, sem.num)
        if last_wait.get(key, -1) >= val:
            return
        last_wait[key] = val
        eng.wait_ge(sem, val)

    ing_val = [[0] * n_islots for _ in range(bloc)]  # input DMAs issued (x16)
    in_need = {}                                     # (m, b) -> s_ing value
    og_val = [[0] * n_oslots for _ in range(bloc)]
    og_after_group = [[0] * bloc for _ in range(ngrp)]
    cpa_count = [0] * bloc  # ACT copy instructions emitted
    cpd_count = [0] * bloc  # DVE copy instructions emitted
    # (b, m_even) -> (sem, value) proving that pair's copy is done
    cp_after_pair = {}

    # ---- weights + chunk-0 inputs (SP ring) ----
    for i in range(2):
        sp.dma_start(wt[:, i, :], gw[i, :, :]).then_inc(s_w, 16)
    for b in range(bloc):
        n0 = min(128, t)
        sp.dma_start(rhs0[b][0:n0, :], x[b, 0:n0, :]).then_inc(s_in0[b], 16)

    # ---- steady-state input DMA groups (SP ring): overlapping 128-row
    # windows, stride C rows between group columns ----
    def issue_in_group(g):
        chunks_g = groups[g]
        slot = g % n_islots
        full = [m for m in chunks_g if chunks[m][3] == 128]
        partial = [m for m in chunks_g if chunks[m][3] < 128]
        nf = len(full)
        for b in range(bloc):
            if g >= n_islots:
                # grhs slot (used by group g-n_islots) reused: all matmuls
                # of that group must be done
                wait(sp, "sp", s_mm[b], groups[g - n_islots][-1] + 2)
            if nf:
                w0 = chunks[full[0]][1]
                src = AP(x[b, :, :].tensor, b * t * f + w0 * f,
                         [[f, 128], [C * f, nf], [1, f]])
                dst = grhs[b][slot][:, 0:nf, :] if nf > 1 else grhs[b][slot][:, 0, :]
                sp.dma_start(dst, src).then_inc(s_ing[b][slot], 16)
                ing_val[b][slot] += 16
            for m in partial:
                _, w0, _, nwin = chunks[m]
                j = m - chunks_g[0]
                sp.dma_start(
                    grhs[b][slot][0:nwin, j, :], x[b, w0 : w0 + nwin, :]
                ).then_inc(s_ing[b][slot], 16)
                ing_val[b][slot] += 16
            for m in chunks_g:
                in_need[(m, b)] = ing_val[b][slot]

    # ---- chunk 0 compute (exact matrix; PSUM bank 0) ----
    for b in range(bloc):
        wait(pe, "pe", s_w, 32)
        wait(pe, "pe", s_in0[b], 16)
        pe.matmul(psp[b][:, 0:f], wt[:, 0, :], rhs0[b][:, :], start=True, stop=True).then_inc(
            s_mm[b], 1
        )
    for b in range(bloc):
        wait(act, "act", s_mm[b], 1)
        act.copy(ot0[b][0:127, :], psp[b][0:127, 0:f]).then_inc(s_cpa[b], 1)
        cpa_count[b] += 1
    for b in range(bloc):
        wait(pool, "pool", s_in0[b], 16)
        pool.dma_start(out[b, 0:1, :], rhs0[b][0:1, :]).then_inc(s_o0[b], 16)
        wait(pool, "pool", s_cpa[b], 1)
        pool.dma_start(out[b, 1:128, :], ot0[b][0:127, :]).then_inc(s_o0[b], 16)

    # prime the input pipeline as deep as the slots allow
    for g in range(min(n_islots, ngrp)):
        issue_in_group(g)

    # ---- steady chunks ----
    for g, chunks_g in enumerate(groups):
        islot = g % n_islots
        oslot = g % n_oslots
        if g + n_islots < ngrp:
            issue_in_group(g + n_islots)

        pairs = [chunks_g[i : i + 2] for i in range(0, len(chunks_g), 2)]
        for pair in pairs:
            # batch-outer emission: both chunks of a batch's pair are
            # adjacent on PE, so its pair copy fires as soon as its own
            # matmuls finish (no cross-batch lockstep on the 2-bank PSUM)
            for b in range(bloc):
                for m in pair:
                    j = m - chunks_g[0]
                    nwin = chunks[m][3]
                    half = m & 1
                    wait(pe, "pe", s_ing[b][islot], in_need[(m, b)])
                    # PSUM bank reuse: the pair containing chunk m-2 (or
                    # chunk0 for m=0, same bank 0) must be copied out
                    if m >= 2:
                        sem, val = cp_after_pair[(b, (m - 2) & ~1)]
                        wait(pe, "pe", sem, val)
                    elif m == 0:
                        wait(pe, "pe", s_cpa[b], 1)
                    pe.matmul(
                        psp[b][:, half * f : (half + 1) * f],
                        wt[0:nwin, 1, :],
                        grhs[b][islot][0:nwin, j, :],
                        start=True,
                        stop=True,
                    ).then_inc(s_mm[b], 1)
            m0 = pair[0]
            j0 = m0 - chunks_g[0]
            npair = len(pair)
            nout_hi = chunks[pair[-1]][2]
            for b in range(bloc):
                # alternate the copy between ACT and DVE so neither engine
                # is the throughput wall; per-engine sems keep completion
                # ordering sound
                use_act = (not use_dve) or ((m0 // 2) + b) & 1 == 0
                ceng, cname, csem, ccnt = (
                    (act, "act", s_cpa, cpa_count)
                    if use_act
                    else (dve, "dve", s_cpd, cpd_count)
                )
                if m0 == chunks_g[0] and g >= n_oslots:
                    # gout slot reused: that group's output DMAs must be done
                    wait(
                        ceng, cname, s_og[b][oslot], og_after_group[g - n_oslots][b]
                    )
                wait(ceng, cname, s_mm[b], pair[-1] + 2)
                if npair == 2 and nout_hi == C:
                    dst = gout[b][oslot][0:C, j0 : j0 + 2, :]
                    src = psp[b][0:C, 0 : 2 * f]
                else:
                    dst = gout[b][oslot][0 : chunks[m0][2], j0, :]
                    src = psp[b][0 : chunks[m0][2], 0:f]
                    if npair == 2:
                        dst2 = gout[b][oslot][0:nout_hi, j0 + 1, :]
                        src2 = psp[b][0:nout_hi, f : 2 * f]
                        if use_act:
                            act.copy(dst, src)
                        else:
                            dve.tensor_copy(dst, src)
                        dst, src = dst2, src2
                if use_act:
                    act.copy(dst, src).then_inc(csem[b], 1)
                else:
                    dve.tensor_copy(dst, src).then_inc(csem[b], 1)
                ccnt[b] += 1
                cp_after_pair[(b, m0)] = (csem[b], ccnt[b])

            # output DMA per pair, fired as soon as the pair's copy lands.
            # Ring split: alternate Pool/SWDGE and ACT/HWDGE rings so no
            # single descriptor-generation path becomes the wall.
            t0p = chunks[m0][0]
            for b in range(bloc):
                sem, val = cp_after_pair[(b, m0)]
                pool_ring = (not ring_split) or ((m0 // 2) + b) & 1 == 0
                ring, rname = (pool, "pool") if pool_ring else (act, "act")
                wait(ring, rname, sem, val)
                if npair == 2 and nout_hi == C:
                    dst = out[b, t0p : t0p + 2 * C, :].rearrange(
                        "(g p) f -> p g f", g=2
                    )
                    src = gout[b][oslot][0:C, j0 : j0 + 2, :]
                    ring.dma_start(dst, src).then_inc(s_og[b][oslot], 16)
                    og_val[b][oslot] += 16
                else:
                    for m in pair:
                        t0m, _, nout, _ = chunks[m]
                        jm = m - chunks_g[0]
                        ring.dma_start(
                            out[b, t0m : t0m + nout, :],
                            gout[b][oslot][0:nout, jm, :],
                        ).then_inc(s_og[b][oslot], 16)
                        og_val[b][oslot] += 16

        for b in range(bloc):
            og_after_group[g][b] = og_val[b][oslot]

    # ---- final: ensure all output DMAs land before program end ----
    for b in range(bloc):
        pool.wait_ge(s_o0[b], 32)
        for slot in range(n_oslots):
            if og_val[b][slot]:
                pool.wait_ge(s_og[b][slot], og_val[b][slot])

    # leave semaphores clean for the next load/execution
    pool.dma_reset(range(lo, hi))
    pool.sem_clear(range(lo, hi))

    return nc


_CACHE = {}


def _get_nc():
    if "nc" not in _CACHE:
        _CACHE["nc"] = build_nc()
    return _CACHE["nc"]


def _run(x, **kwargs):
    x16 = np.ascontiguousarray(np.asarray(x), dtype=np.float16)
    assert x16.shape == (B, T, F), x16.shape
    nc = _get_nc()
    gwv = _build_gmats()
    in_maps = [
        {"x": np.ascontiguousarray(x16[c * BLOC : (c + 1) * BLOC]), "gw": gwv}
        for c in range(NCORES)
    ]
    res = run_bass_kernel_spmd(nc, in_maps, core_ids=list(range(NCORES)), **kwargs)
    out = np.concatenate([res.results[c]["out"] for c in range(NCORES)], axis=0)
    return out.astype(np.float32), res


def kernel(x):
    return _run(x)[0]
